# revision 5
# baseline (speedup 1.0000x reference)
"""CTC focal loss on 8 Trainium2 NeuronCores (Bass/Tile).

Strategy: data-parallel over the batch (16 rows per core). Per core, the
T-step CTC forward DP runs in the log domain as a 3-way stabilized
log-sum-exp per state. Layout: partition p = group*16 + row, where the 408
(padded) extended states are split into 8 groups of 51; each group also
recomputes R redundant lower states so the cross-group boundary only needs
an SBUF->SBUF DMA every K steps. Per-row "collector" states end+1/end+2
(driven by a host-crafted log-prob schedule) capture logaddexp(a[end],
a[end-1]) at exactly t = preds_len and latch it to the end of the loop, so
the final loss is read from the last alpha tile with no mid-loop control
flow.
"""
from contextlib import ExitStack

import numpy as np

import concourse.bass as bass
import concourse.bacc as bacc
import concourse.mybir as mybir
import concourse.tile as tile
from concourse.bass_utils import run_bass_kernel_spmd

# problem shape (hardcoded per spec)
T, N, C, L = 2048, 128, 96, 200
S = 2 * L + 1          # 401 real extended states
SG = 51                # states per group (8 * 51 = 408 >= S + collectors)
G = 8                  # state groups
NROW = 16              # batch rows per core
NCORES = 8
P = 128                # partitions = G * NROW

NEG0 = np.float32(-30000.0)
GAMMA = 2.0
ALPHA = 1.0

# schedule
K_EX = 8               # boundary exchange period (steps)
R_RED = 2 * K_EX + 2   # redundant lower states per group
U_UNROLL = 48          # steps per hardware-loop body (K_EX must divide it)
T_DEV = 2064           # total device steps (>= T + 2, multiple of U_UNROLL)
NCH = T_DEV // U_UNROLL

W = SG + R_RED         # computed states per group   (69)
TW = W + 2             # tile width incl 2 pad cols  (71)
CATW = 3 * W           # exp concat width            (207)

_DT = mybir.dt.float32


def _build_nc():
    nc = bacc.Bacc("TRN2", target_bir_lowering=False, debug=False, num_devices=1)
    lp_ap = nc.dram_tensor("lp", [P, NCH * U_UNROLL * W], _DT, kind="ExternalInput").ap()
    mn_ap = nc.dram_tensor("mneg", [P, W], _DT, kind="ExternalInput").ap()
    a0_ap = nc.dram_tensor("a0", [P, TW], _DT, kind="ExternalInput").ap()
    out_ap = nc.dram_tensor("aout", [P, TW], _DT, kind="ExternalOutput").ap()

    add = mybir.AluOpType.add
    mx = mybir.AluOpType.max
    sub = mybir.AluOpType.subtract

    with tile.TileContext(nc) as tc:
        with ExitStack() as ctx:
            const_pool = ctx.enter_context(tc.tile_pool(name="const", bufs=1))
            state_pool = ctx.enter_context(tc.tile_pool(name="state", bufs=1))
            lp_pool = ctx.enter_context(tc.tile_pool(name="lp", bufs=3))
            tmp_pool = ctx.enter_context(tc.tile_pool(name="tmp", bufs=2))

            mn = const_pool.tile([P, W], _DT)
            nc.sync.dma_start(mn[:], mn_ap[:])
            A = state_pool.tile([P, TW], _DT)
            nc.sync.dma_start(A[:], a0_ap[:])
            A2 = state_pool.tile([P, TW], _DT)
            nc.sync.dma_start(A2[:], a0_ap[:])

            tiles = [A, A2]

            with tc.For_i(0, NCH, 1, hint_engines=(mybir.EngineType.DVE,),
                          staggered_reset=True) as ci:
                lpt = lp_pool.tile([P, U_UNROLL * W], _DT)
                nc.sync.dma_start(lpt[:], lp_ap[:, bass.ts(ci, U_UNROLL * W)])
                for u in range(U_UNROLL):
                    src = tiles[u % 2]
                    dst = tiles[1 - (u % 2)]

                    # t3 = a[s-2] + mneg     (Pool engine)
                    t3 = tmp_pool.tile([P, W], _DT, tag="t3")
                    nc.gpsimd.tensor_tensor(t3[:], src[:, 0:W], mn[:], add)
                    # m1 = max(a[s], a[s-1]) ; mm = max(m1, t3)
                    m1 = tmp_pool.tile([P, W], _DT, tag="m1")
                    nc.vector.tensor_tensor(m1[:], src[:, 2:TW], src[:, 1:TW - 1], mx)
                    mm = tmp_pool.tile([P, W], _DT, tag="mm")
                    nc.vector.tensor_tensor(mm[:], m1[:], t3[:], mx)

                    # cat[:, 0:2W]  = [a[s] | a[s-1]] - mm   (2-view AP, bcast mm)
                    # cat[:, 2W:3W] = t3 - mm
                    cat = tmp_pool.tile([P, CATW], _DT, tag="cat")
                    in0 = src[:, 2:TW].copy()
                    pdim = [list(d) for d in list(in0.ap)][0]
                    in0.ap = mybir.VecI64Pair([pdim, [-1, 2], [1, W]])
                    in1 = mm[:, 0:W].unsqueeze(1).broadcast_to([P, 2, W])
                    nc.vector.tensor_tensor(cat[:, 0:2 * W], in0, in1, sub)
                    nc.vector.tensor_tensor(cat[:, 2 * W:CATW], t3[:], mm[:], sub)

                    # e = exp(cat)
                    ecat = tmp_pool.tile([P, CATW], _DT, tag="ecat")
                    nc.scalar.activation(ecat[:], cat[:], mybir.ActivationFunctionType.Exp)

                    # r = e0 + e1 + e2 ; l = ln(r)
                    r1 = tmp_pool.tile([P, W], _DT, tag="r1")
                    nc.vector.tensor_tensor(r1[:], ecat[:, 0:W], ecat[:, W:2 * W], add)
                    r2 = tmp_pool.tile([P, W], _DT, tag="r2")
                    nc.vector.tensor_tensor(r2[:], r1[:], ecat[:, 2 * W:CATW], add)
                    lt = tmp_pool.tile([P, W], _DT, tag="lt")
                    nc.scalar.activation(lt[:], r2[:], mybir.ActivationFunctionType.Ln)

                    # mlp = mm + lp_t ; a'[s] = mlp + l
                    mlp = tmp_pool.tile([P, W], _DT, tag="mlp")
                    nc.vector.tensor_tensor(mlp[:], mm[:], lpt[:, u * W:(u + 1) * W], add)
                    nc.vector.tensor_tensor(dst[:, 2:TW], mlp[:], lt[:], add)

                    if (u + 1) % K_EX == 0:
                        # full refresh of dst's pads+redundant region. The other
                        # tile needs none: its region is recomputed from this
                        # one next step, and corruption entering from its stale
                        # pads climbs 2 states/step -- bounded by R_RED before
                        # the next refresh resets it.
                        nc.sync.dma_start(dst[16:128, 0:R_RED + 2], dst[0:112, SG:TW])

            # U_UNROLL is even, so every body ends with dst = tiles[0]
            nc.sync.dma_start(out_ap[:], tiles[0][:])

    nc.compile()
    return nc


def _host_prepare(predicts, labels, preds_lengths, label_lengths):
    """Build per-core device inputs. predicts (T,N,C) f32 log-probs."""
    predicts = np.ascontiguousarray(predicts, dtype=np.float32)
    labels = np.asarray(labels).astype(np.int64)
    preds_lengths = np.asarray(preds_lengths).astype(np.int64)
    label_lengths = np.asarray(label_lengths).astype(np.int64)

    SP = G * SG  # 408
    ext = np.zeros((N, SP), dtype=np.int64)
    ext[:, 1:S:2] = labels
    skip = np.zeros((N, SP), dtype=bool)
    skip[:, :S] = (ext[:, :S] != 0) & np.concatenate(
        [np.zeros((N, 2), bool), ext[:, 2:S] != ext[:, :S - 2]], axis=1)
    end_idx = 2 * label_lengths            # (N,)

    # collector overrides: state end+1 absorbs (end, end-1) at t*+1 and state
    # end+2 latches it from t*+2 on.
    skip[np.arange(N), end_idx + 1] = True    # allow end-1 -> end+1
    skip[np.arange(N), end_idx + 2] = False   # keep end -> end+2 closed

    in_maps = []
    metas = []
    for c in range(NCORES):
        rows = slice(c * NROW, (c + 1) * NROW)
        lab_rows = np.arange(c * NROW, (c + 1) * NROW)
        # lp_ext[t, i, s] = predicts[t, rows[i], ext[rows[i], s]]
        lp_ext = np.full((T_DEV, NROW, SP), NEG0, dtype=np.float32)
        lp_ext[:T] = predicts[:, lab_rows[:, None], ext[lab_rows]]

        # collector schedules
        e = end_idx[lab_rows]
        tstar = preds_lengths[lab_rows] - 1
        for i in range(NROW):
            lp_ext[:, i, e[i] + 1] = NEG0
            lp_ext[:, i, e[i] + 2] = NEG0
            cap = tstar[i] + 1
            lp_ext[cap, i, e[i] + 1] = 0.0
            lp_ext[cap + 1:, i, e[i] + 2] = 0.0

        # pack to (P, NCH*U*W): p = g*16 + i, col = t*W + w, state = 51g - R + w
        lp_pack = np.full((P, T_DEV, W), NEG0, dtype=np.float32)
        mneg = np.full((P, W), NEG0, dtype=np.float32)
        a0 = np.full((P, TW), NEG0, dtype=np.float32)
        for g in range(G):
            s_lo = SG * g - R_RED
            w_lo = max(0, -s_lo)
            s0 = s_lo + w_lo
            s1 = SG * g + SG
            lp_pack[g * NROW:(g + 1) * NROW, :, w_lo:] = \
                lp_ext[:, :, s0:s1].transpose(1, 0, 2)
            m = np.where(skip[lab_rows, s0:s1], np.float32(0.0), NEG0)
            mneg[g * NROW:(g + 1) * NROW, w_lo:] = m
        # init alpha: state 0 = 0.0 at group 0 col R+2
        a0[0:NROW, R_RED + 2] = 0.0

        in_maps.append({
            "lp": np.ascontiguousarray(lp_pack.reshape(P, T_DEV * W)),
            "mneg": mneg,
            "a0": a0,
        })
        metas.append({"end_idx": e, "rows": lab_rows})
    return in_maps, metas


def _host_finish(results, metas):
    total = np.float64(0.0)
    for res, meta in zip(results, metas):
        aout = res["aout"]  # (P, TW)
        e = meta["end_idx"]
        for i in range(NROW):
            s = e[i] + 2                    # latch state
            g = s // SG
            col = s - (SG * g - R_RED) + 2
            final = np.float64(aout[g * NROW + i, col])
            ctc = -final
            w = ALPHA * (1.0 - np.exp(-ctc)) ** GAMMA
            total += ctc * w
    return np.float32(total)


_NC_CACHE = None


def kernel(predicts, labels, ref_labels, preds_lengths, label_lengths, ref_length):
    global _NC_CACHE
    if _NC_CACHE is None:
        _NC_CACHE = _build_nc()
    nc = _NC_CACHE
    in_maps, metas = _host_prepare(predicts, labels, preds_lengths, label_lengths)
    out = run_bass_kernel_spmd(nc, in_maps, list(range(NCORES)))
    return _host_finish(out.results, metas)
